# revision 1
# baseline (speedup 1.0000x reference)
"""Trainium2 Bass kernel for nn_DocREModel (segment_reduce / DocRE relation extraction).

Strategy (8 NeuronCores, data-parallel over documents):
  - core c handles doc b = c//2, half h = c%2 of that doc's deduplicated
    (head, tail) entity-pair combos (padded to NQ=384 per core).
  - per core, on device: entity logsumexp embeddings, mention-averaged entity
    attentions, one-hot-matmul gathers of head/tail attention rows,
    rs = sum_h(Ha*Ta) normalized, ctx = rs @ seq (PE, contraction over L),
    head-extractor matmuls + tanh, grouped-bilinear outer products (DVE
    broadcast-AP) -> PE transposes -> W_bl k-chunk matmuls accumulating
    logitsT[97, NQ] in PSUM.
  - host: index prep (dedup, one-hots, mention gather), shard, and scatter of
    per-combo logits back to the 3968 pair rows.
"""

import numpy as np
import ml_dtypes

import concourse.bass as bass
import concourse.mybir as mybir
import concourse.tile as tile
from concourse import bacc
from concourse.bass_utils import run_bass_kernel_spmd
from concourse.masks import make_identity

BF16 = mybir.dt.bfloat16
F32 = mybir.dt.float32
AF = mybir.ActivationFunctionType
ALU = mybir.AluOpType
AX = mybir.AxisListType

SMALL_NEG = -10000000000.0
BS, L, H, HEADS = 4, 1024, 768, 12
E, M, R = 32, 8, 992
EMB, BLOCK, NCLS = 768, 64, 97
GRP = EMB // BLOCK          # 12 bilinear groups
P = BS * R                  # 3968 pairs
KCH = EMB * BLOCK // 128    # 384 classifier k-chunks
LC = L // 128               # 8 l-chunks

NP_BF16 = ml_dtypes.bfloat16

_NC_CACHE: dict[int, bacc.Bacc] = {}


def _build(
    NQ: int, timing_mode: bool = False, nrep: int = 1, marker: float = 0.0
) -> bacc.Bacc:
    """Build + compile the per-core Bass program (combos padded to NQ).

    timing_mode: big inputs become device-resident Internal DRAM (garbage
    data, no per-call upload over axon) and the body repeats `nrep` times —
    used only to calibrate HW exec time via wall-clock deltas.
    """
    assert NQ % 128 == 0
    NCH = NQ // 128

    nc = bacc.Bacc("TRN2", target_bir_lowering=False, debug=False)

    big = "Internal" if timing_mode else "ExternalInput"
    meD = nc.dram_tensor("meD", [128, 2, H], F32, kind="ExternalInput")
    mattD = nc.dram_tensor("mattD", [2, 128, HEADS, L], BF16, kind=big)
    ohH = nc.dram_tensor("ohH", [128, NQ], BF16, kind=big)
    ohT = nc.dram_tensor("ohT", [128, NQ], BF16, kind=big)
    seqT = nc.dram_tensor("seqT", [LC, 128, H], BF16, kind=big)
    W1D = nc.dram_tensor("W1", [128, E], BF16, kind=big)
    W2D = nc.dram_tensor("W2", [2, 128, E], BF16, kind=big)
    WhD = nc.dram_tensor("Wh", [12, 128, EMB], BF16, kind=big)
    bhD = nc.dram_tensor("bh", [6, 128, 1], F32, kind=big)
    WblD = nc.dram_tensor("Wbl", [128, KCH, NCLS], BF16, kind=big)
    bblD = nc.dram_tensor("bbl", [NCLS, 1], F32, kind=big)
    logD = nc.dram_tensor("logT", [NCLS, NQ], F32, kind="ExternalOutput")

    with tile.TileContext(nc) as tc:
      for _rep in range(nrep):
        with tc.tile_pool(name="persist", bufs=1) as pp:
            # persistent SBUF tensors
            # tiles for weights used only in phases C-E; their DMAs are
            # emitted after phase B so they don't delay the mattD stream
            Wh = pp.tile([128, 12, EMB], BF16)
            bh = pp.tile([128, 6], F32)
            bbl = pp.tile([NCLS, 1], F32)
            seq_sb = pp.tile([128, LC, H], BF16)
            oh_h = pp.tile([128, NQ], BF16)
            nc.sync.dma_start(oh_h[:], ohH.ap()[:])
            oh_t = pp.tile([128, NQ], BF16)
            nc.sync.dma_start(oh_t[:], ohT.ap()[:])
            W1 = pp.tile([128, E], BF16)
            nc.sync.dma_start(W1[:], W1D.ap()[:])
            W2 = pp.tile([128, 2, E], BF16)
            nc.sync.dma_start(W2[:], W2D.ap()[:].rearrange("k p m -> p k m"))
            ident = pp.tile([128, 128], BF16)
            make_identity(nc, ident[:])

            EE = pp.tile([E, EMB], BF16)           # entity embeddings (logsumexp)
            A_sb = pp.tile([128, HEADS, 256], BF16)  # entity attns [lq*32+e, h, lm]
            rsT = pp.tile([128, LC, NQ], BF16)     # transposed normalized rs
            XTh = pp.tile([128, 12, NQ], BF16)     # [hsT; ctxT] k-chunks
            XTt = pp.tile([128, 6, NQ], BF16)      # tsT k-chunks (ctx shared w/ XTh)
            hsET = pp.tile([128, 6, NQ], BF16)     # tanh head-extractor out (emb-part)
            tsET = pp.tile([128, 6, NQ], BF16)
            hsE = pp.tile([128, NCH, EMB], BF16)   # pair-partition orientation
            tsE = pp.tile([128, NCH, EMB], BF16)
            # hs with every element duplicated (hd[2k]=hd[2k+1]=hs[k]) so the
            # bilinear outer-product TT reads unit-stride pairs -> DVE 2x mode
            hsD = pp.tile([128, NCH, 2 * EMB], BF16)

            # ---------------- Phase A1: entity embeddings -------------------
            with (
                tc.tile_pool(name="pa", bufs=1) as pa,
                tc.tile_pool(name="pa2", bufs=4) as pa2,
                tc.tile_pool(name="psA", bufs=1, space="PSUM") as psA,
                tc.tile_pool(name="pb", bufs=4) as pb,
                tc.tile_pool(name="psB", bufs=2, space="PSUM") as psB,
            ):
                me = pa.tile([128, 2, H], F32, tag="me")
                nc.sync.dma_start(me[:], meD.ap()[:])
                e0 = pa.tile([128, H], F32, tag="e0")
                e1 = pa.tile([128, H], F32, tag="e1")
                nc.scalar.activation(e0[:], me[:, 0, :], AF.Exp)
                nc.scalar.activation(e1[:], me[:, 1, :], AF.Exp)
                s1 = pa.tile([128, H], F32, tag="s1")
                nc.vector.tensor_add(s1[:], e0[:], e1[:])
                s1b = pa.tile([128, H], BF16, tag="s1b")
                nc.vector.tensor_copy(s1b[:], s1[:])
                eps = psA.tile([E, H], F32, tag="eps")
                nc.tensor.matmul(eps[:, 0:512], W1[:], s1b[:, 0:512])
                nc.tensor.matmul(eps[:, 512:768], W1[:], s1b[:, 512:768])
                nc.scalar.activation(EE[:], eps[:], AF.Ln)

                # ------------ Phase A2: entity attentions -------------------
                for s in range(6):  # n-slices of 512 = (2 heads, 256)
                    psa = psA.tile([128, 512], F32, tag="psa")
                    for lq in range(4):
                        mts = []
                        for kc in range(2):
                            mt = pa2.tile([128, 2, 256], BF16, tag=f"mt{kc}")
                            nc.sync.dma_start(
                                mt[:],
                                mattD.ap()[kc][
                                    :, 2 * s : 2 * s + 2, 256 * lq : 256 * (lq + 1)
                                ],
                            )
                            mts.append(mt)
                        for kc in range(2):
                            nc.tensor.matmul(
                                psa[32 * lq : 32 * (lq + 1), :],
                                W2[:, kc, :],
                                mts[kc][:].rearrange("p a b -> p (a b)"),
                                start=(kc == 0),
                                stop=(kc == 1),
                                tile_position=(0, 32 * lq),
                            )
                    nc.vector.tensor_copy(
                        A_sb[:, 2 * s : 2 * s + 2, :].rearrange("p a b -> p (a b)"),
                        psa[:],
                    )

                # ------------ Phase B: gathers + rs (per q-chunk) -----------
                # (same pool scope as A so B's PSUM/SBUF tiles don't reuse
                # A's banks — pool-boundary reuse would serialize B behind A)
                for qc in range(NCH):
                    qsl = slice(128 * qc, 128 * (qc + 1))
                    rs = pb.tile([128, 4, 256], BF16, tag="rs")
                    for lq in range(4):
                        esl = slice(32 * lq, 32 * (lq + 1))
                        prod = pb.tile([128, HEADS, 256], BF16, tag="prod")
                        for hp in range(6):
                            hps = psB.tile([128, 512], F32, tag="hps")
                            nc.tensor.matmul(
                                hps[:],
                                oh_h[esl, qsl],
                                A_sb[esl, 2 * hp : 2 * hp + 2, :],
                                tile_position=(32 * lq, 0),
                            )
                            hsb = pb.tile([128, 512], BF16, tag="hsb")
                            nc.scalar.activation(hsb[:], hps[:], AF.Copy)
                            tps = psB.tile([128, 512], F32, tag="tps")
                            nc.tensor.matmul(
                                tps[:],
                                oh_t[esl, qsl],
                                A_sb[esl, 2 * hp : 2 * hp + 2, :],
                                tile_position=(32 * lq, 0),
                            )
                            po = prod[:, 2 * hp : 2 * hp + 2, :].rearrange(
                                "p a b -> p (a b)"
                            )
                            if hp % 2 == 0:
                                # DVE mul reading T straight from PSUM (1x)
                                nc.vector.scalar_tensor_tensor(
                                    po, tps[:], 1.0, hsb[:], ALU.mult, ALU.mult
                                )
                            else:
                                # ACT evacuates T too; DVE mul runs 2x on bf16
                                tsb = pb.tile([128, 512], BF16, tag="tsb")
                                nc.scalar.activation(tsb[:], tps[:], AF.Copy)
                                nc.vector.tensor_tensor(po, tsb[:], hsb[:], ALU.mult)
                        # reduce over 12 heads: wide pair-add tree
                        t6 = pb.tile([128, 6, 256], BF16, tag="t6")
                        nc.vector.tensor_add(
                            t6[:], prod[:, 0:6, :], prod[:, 6:12, :]
                        )
                        t3 = pb.tile([128, 3, 256], BF16, tag="t3")
                        nc.vector.tensor_add(t3[:], t6[:, 0:3, :], t6[:, 3:6, :])
                        t1 = pb.tile([128, 256], BF16, tag="t1")
                        nc.vector.tensor_add(t1[:], t3[:, 0, :], t3[:, 1, :])
                        nc.vector.tensor_add(rs[:, lq, :], t1[:], t3[:, 2, :])
                    # normalize rows of rs [128, 1024]
                    rsum = pb.tile([128, 1], F32, tag="rsum")
                    nc.vector.tensor_reduce(
                        rsum[:], rs[:].rearrange("p a b -> p (a b)"), AX.X, ALU.add
                    )
                    rcp = pb.tile([128, 1], F32, tag="rcp")
                    nc.vector.reciprocal(rcp[:], rsum[:])
                    rsn = pb.tile([128, 1024], BF16, tag="rsn")
                    nc.vector.tensor_scalar(
                        rsn[:], rs[:].rearrange("p a b -> p (a b)"), rcp[:], None,
                        ALU.mult,
                    )
                    # transpose rs -> rsT[l, q]
                    for lc in range(LC):
                        tp = psB.tile([128, 128], BF16, tag="tp", bufs=1)
                        nc.tensor.transpose(
                            tp[:], rsn[:, 128 * lc : 128 * (lc + 1)], ident[:]
                        )
                        nc.vector.tensor_copy(rsT[:, lc, qsl], tp[:])

            # deferred weight loads (stream during phases A/B)
            nc.sync.dma_start(seq_sb[:], seqT.ap()[:].rearrange("k p m -> p k m"))
            nc.sync.dma_start(Wh[:], WhD.ap()[:].rearrange("k p m -> p k m"))
            nc.sync.dma_start(bh[:], bhD.ap()[:].rearrange("k p o -> p (k o)"))
            nc.sync.dma_start(bbl[:], bblD.ap()[:])

            # ---------------- Phase C: ctx matmuls (ctxT into XTh) ----------
            with tc.tile_pool(name="psC", bufs=2, space="PSUM") as psC:
                for mc in range(6):
                    cps = psC.tile([128, NQ], F32, tag="cps", bufs=3)
                    for lc in range(LC):
                        nc.tensor.matmul(
                            cps[:],
                            seq_sb[:, lc, 128 * mc : 128 * (mc + 1)],
                            rsT[:, lc, :],
                            start=(lc == 0),
                            stop=(lc == LC - 1),
                        )
                    nc.vector.tensor_copy(XTh[:, 6 + mc, :], cps[:])

                # hsT / tsT gathers from EE
                for mc in range(6):
                    gps = psC.tile([128, NQ], F32, tag="gps")
                    nc.tensor.matmul(
                        gps[:], EE[:, 128 * mc : 128 * (mc + 1)], oh_h[0:E, :]
                    )
                    nc.vector.tensor_copy(XTh[:, mc, :], gps[:])
                    gps2 = psC.tile([128, NQ], F32, tag="gps2")
                    nc.tensor.matmul(
                        gps2[:], EE[:, 128 * mc : 128 * (mc + 1)], oh_t[0:E, :]
                    )
                    nc.vector.tensor_copy(XTt[:, mc, :], gps2[:])

            # ---------------- Phase D: head extractor + transposes ----------
            with tc.tile_pool(name="psD", bufs=4, space="PSUM") as psD:
                for side, dst in ((0, hsET), (1, tsET)):
                    for mc in range(6):
                        dps = psD.tile([128, NQ], F32, tag="dps")
                        for kc in range(12):
                            if kc < 6 and side == 1:
                                rhs = XTt[:, kc, :]
                            else:
                                rhs = XTh[:, kc, :]
                            nc.tensor.matmul(
                                dps[:],
                                Wh[:, kc, 128 * mc : 128 * (mc + 1)],
                                rhs,
                                start=(kc == 0),
                                stop=(kc == 11),
                            )
                        nc.scalar.activation(
                            dst[:, mc, :], dps[:], AF.Tanh, bias=bh[:, mc : mc + 1]
                        )
                # transpose to pair-partition orientation (qc-outer so phase E
                # can start on qc=0 while later chunks still transpose)
                for qc in range(NCH):
                    for src, dst in ((hsET, hsE), (tsET, tsE)):
                        for mc in range(6):
                            tp2 = psD.tile([128, 128], BF16, tag="tp2")
                            nc.tensor.transpose(
                                tp2[:], src[:, mc, 128 * qc : 128 * (qc + 1)], ident[:]
                            )
                            nc.vector.tensor_copy(
                                dst[:, qc, 128 * mc : 128 * (mc + 1)], tp2[:]
                            )
                    nc.scalar.activation(
                        hsD[:, qc, :].rearrange("p (k l) -> p k l", l=2),
                        hsE[:, qc, :].unsqueeze(2).broadcast_to([128, EMB, 2]),
                        AF.Copy,
                    )

            # ---------------- Phase E: bilinear + classifier ----------------
            with (
                tc.tile_pool(name="pe", bufs=3) as pe,
                tc.tile_pool(name="psE", bufs=1, space="PSUM") as psE,
                tc.tile_pool(name="psEt", bufs=4, space="PSUM") as psEt,
            ):
                lps = psE.tile([NCLS, NQ], F32)
                for g in range(GRP):
                    gsl = slice(BLOCK * g, BLOCK * (g + 1))
                    Wblg = pe.tile([128, 32, NCLS], BF16, tag="wblg", bufs=3)
                    nc.sync.dma_start(Wblg[:], WblD.ap()[:, 32 * g : 32 * (g + 1), :])
                    bls = []
                    for qc in range(NCH):
                        bl = pe.tile([128, BLOCK * BLOCK], BF16, tag=f"bl{qc}")
                        # out (i, jh, jl): all three operands end in a
                        # unit-stride pair dim -> DVE 2x_1p mode
                        in0 = (
                            hsD[:, qc, 2 * BLOCK * g : 2 * BLOCK * (g + 1)]
                            .rearrange("p (i l) -> p i l", l=2)
                            .unsqueeze(2)
                            .broadcast_to([128, BLOCK, BLOCK // 2, 2])
                        )
                        in1 = (
                            tsE[:, qc, gsl]
                            .rearrange("p (jh l) -> p jh l", l=2)
                            .unsqueeze(1)
                            .broadcast_to([128, BLOCK, BLOCK // 2, 2])
                        )
                        nc.vector.tensor_tensor(
                            bl[:].rearrange(
                                "p (i jh l) -> p i jh l", i=BLOCK, l=2
                            ),
                            in0,
                            in1,
                            ALU.mult,
                        )
                        bls.append(bl)
                    for tp2 in range(16):  # two k-chunks per PSUM tile / copy
                        blt = pe.tile([128, 2, NQ], BF16, tag="blt", bufs=8)
                        btp = psEt.tile([128, 2, NQ], BF16, tag="btp", bufs=6)
                        for ti in range(2):
                            t = 2 * tp2 + ti
                            for qc in range(NCH):
                                nc.tensor.transpose(
                                    btp[:, ti, 128 * qc : 128 * (qc + 1)],
                                    bls[qc][:, 128 * t : 128 * (t + 1)],
                                    ident[:],
                                )
                        if tp2 % 8 < 3:  # ~40% on DVE (2x mode), rest ACT
                            nc.vector.tensor_copy(blt[:], btp[:])
                        else:
                            nc.scalar.activation(
                                blt[:].rearrange("p a b -> p (a b)"),
                                btp[:].rearrange("p a b -> p (a b)"),
                                AF.Copy,
                            )
                        for ti in range(2):
                            k = 32 * g + 2 * tp2 + ti
                            nc.tensor.matmul(
                                lps[:],
                                Wblg[:, 2 * tp2 + ti, :],
                                blt[:, ti, :],
                                start=(k == 0),
                                stop=(k == KCH - 1),
                            )
                lsb = pe.tile([NCLS, NQ], F32, tag="lsb")
                nc.vector.tensor_scalar(lsb[:], lps[:], bbl[:], None, ALU.add)
                if marker:
                    nc.scalar.add(lsb[:], lsb[:], marker)
                nc.sync.dma_start(logD.ap()[:], lsb[:])

    nc.compile()
    return nc


def _get_nc(NQ: int) -> bacc.Bacc:
    if NQ not in _NC_CACHE:
        _NC_CACHE[NQ] = _build(NQ)
    return _NC_CACHE[NQ]


def _host_prep(inputs: dict, NQ: int):
    """Build per-core input maps + output scatter info."""
    seq_embs = np.asarray(inputs["seq_embs"], np.float32)
    attentions = np.asarray(inputs["attentions"], np.float32)
    entity_pos = np.asarray(inputs["entity_pos"], np.int32)
    hts = np.asarray(inputs["hts"], np.int32)
    W_head = np.asarray(inputs["W_head"], np.float32)
    b_head = np.asarray(inputs["b_head"], np.float32)
    W_bl = np.asarray(inputs["W_bl"], np.float32)
    b_bl = np.asarray(inputs["b_bl"], np.float32)

    # shared constant tensors
    Wh = np.ascontiguousarray(
        W_head.reshape(12, 128, EMB).astype(NP_BF16)
    )
    bh = np.ascontiguousarray(b_head.reshape(6, 128, 1).astype(np.float32))
    Wbl = np.ascontiguousarray(
        W_bl.reshape(KCH, 128, NCLS).transpose(1, 0, 2).astype(NP_BF16)
    )
    bbl = np.ascontiguousarray(b_bl.reshape(NCLS, 1).astype(np.float32))
    W1 = np.zeros((128, E), NP_BF16)
    for e in range(E):
        W1[4 * e : 4 * e + 4, e] = 1.0

    in_maps = []
    scatter = []  # per core: (global pair rows, combo position per row)
    for b in range(BS):
        pos = entity_pos[E * b : E * (b + 1)]          # [32, 8]
        mask = pos >= 0
        n_ment = mask.sum(1)
        pc = np.where(mask, pos, 0)

        me = seq_embs[b][pc]                            # [32, 8, H]
        me[~mask] = SMALL_NEG
        meD = np.ascontiguousarray(
            me.reshape(E, 4, 2, H).reshape(128, 2, H).astype(np.float32)
        )

        ma = attentions[b].transpose(1, 0, 2)[pc.reshape(-1)]  # [256, 12, L]
        ma[~mask.reshape(-1)] = 0.0
        mattD = np.ascontiguousarray(ma.reshape(2, 128, HEADS, L).astype(NP_BF16))

        W2 = np.zeros((2, 128, E), np.float32)
        for e in range(E):
            kc, el = divmod(e, 16)
            W2[kc, 8 * el : 8 * el + 8, e] = mask[e] / n_ment[e]
        W2 = W2.astype(NP_BF16)

        seqT = np.ascontiguousarray(
            seq_embs[b].reshape(LC, 128, H).astype(NP_BF16)
        )

        # dedup combos for this doc
        ht = hts[R * b : R * (b + 1)]
        keys = ht[:, 0] * E + ht[:, 1]
        uq, inv = np.unique(keys, return_inverse=True)
        D = len(uq)
        n0 = min((D + 1) // 2, NQ)
        assert D <= 2 * NQ, f"doc {b}: {D} distinct combos > capacity {2 * NQ}"
        halves = (uq[:n0], uq[n0:])
        for hf in range(2):
            u = halves[hf]
            heads = (u // E).astype(np.int64)
            tails = (u % E).astype(np.int64)
            nq = len(u)
            heads = np.concatenate([heads, np.zeros(NQ - nq, np.int64)])
            tails = np.concatenate([tails, np.zeros(NQ - nq, np.int64)])
            ohh = np.zeros((128, NQ), np.float32)
            oht = np.zeros((128, NQ), np.float32)
            for lq in range(4):
                ohh[32 * lq + heads, np.arange(NQ)] = 1.0
                oht[32 * lq + tails, np.arange(NQ)] = 1.0
            in_maps.append(
                {
                    "meD": meD, "mattD": mattD,
                    "ohH": ohh.astype(NP_BF16), "ohT": oht.astype(NP_BF16),
                    "seqT": seqT, "W1": W1, "W2": W2,
                    "Wh": Wh, "bh": bh, "Wbl": Wbl, "bbl": bbl,
                }
            )
        # scatter info: pair row r of doc b -> (core, position)
        rows = R * b + np.arange(R)
        core = 2 * b + (inv >= n0).astype(np.int64)
        posn = np.where(inv < n0, inv, inv - n0)
        scatter.append((rows, core, posn))
    return in_maps, scatter


def kernel(**inputs) -> np.ndarray:
    hts = np.asarray(inputs["hts"], np.int32)
    # capacity: NQ per core = half a doc's distinct combos, padded to 128
    maxD = 0
    for b in range(BS):
        ht = hts[R * b : R * (b + 1)]
        maxD = max(maxD, len(np.unique(ht[:, 0] * E + ht[:, 1])))
    NQ = max(384, ((maxD + 1) // 2 + 127) // 128 * 128)

    in_maps, scatter = _host_prep(inputs, NQ)
    nc = _get_nc(NQ)
    last_err = None
    for _attempt in range(3):
        try:
            res = run_bass_kernel_spmd(nc, in_maps, core_ids=list(range(8)))
            break
        except Exception as e:  # transient device wedge (e.g. NRT_EXEC_UNIT_*)
            last_err = e
    else:
        raise last_err

    logits = np.empty((P, NCLS), np.float32)
    lts = [res.results[c]["logT"] for c in range(8)]
    for rows, core, posn in scatter:
        for hf in range(2):
            m = core == rows[0] // R * 2 + hf
            if m.any():
                logits[rows[m]] = lts[rows[0] // R * 2 + hf][:, posn[m]].T
    return logits

